# revision 4
# baseline (speedup 1.0000x reference)
"""LocalRNN (sliding-window GRU) Trainium2 Bass kernel.

x: [8, 1024, 512] f32, window K=7, GRU hidden 512. Output [8, 1024, 512].

Strategy: one batch element per NeuronCore (8 cores). Per core, feature-major
layout [feature, token]: the K-step recurrence needs h as matmul rhs in
[d, tok] layout, and gate elementwise ops run on [128, ntok] tiles. The
input projection gx = W_ih @ x is computed once for all tokens; window step k
reads gx shifted by k columns (left zero-padding becomes 6 zero columns).
"""

import json
import numpy as np

import concourse.bass as bass
import concourse.mybir as mybir
import concourse.tile as tile
from concourse import bass_utils as _bu
from concourse import bass2jax as _b2j

B, L, D, K = 8, 1024, 512, 7
G = 3 * D          # 1536 gate dim
NC = D // 128      # 4 feature chunks
MC = G // 128      # 12 gate chunks
NT = 2             # token chunks of 512
TCH = L // NT      # 512 tokens per chunk
FP = mybir.dt.float32

# ---------------------------------------------------------------------------
# Workaround: this walrus build rejects instructions with >2 semaphore waits
# ("Too many sync wait commands"). Split excess waits onto injected NoOps on
# the same engine immediately before the instruction.
def _split_excess_waits(bir: bytes) -> bytes:
    m = json.loads(bir)
    n_new = 0
    for fn in m.get("functions", []):
        for blk in fn.get("blocks", []):
            insts = blk.get("instructions", [])
            out = []
            for inst in insts:
                si = inst.get("sync_info")
                waits = si.get("on_wait") if si else None
                lim = 1
                if waits and len(waits) > lim:
                    extra = waits[:-lim]
                    inst["sync_info"]["on_wait"] = waits[-lim:]
                    for i in range(0, len(extra), 1):
                        n_new += 1
                        out.append({
                            "debug": inst.get("debug", 0),
                            "engine": inst["engine"],
                            "ins": [],
                            "name": f"{inst['name']}w{n_new}",
                            "opcode": "NoOp",
                            "outs": [],
                            "sync_info": {
                                "on_update": [],
                                "on_wait": extra[i:i + 1],
                            },
                        })
                out.append(inst)
            blk["instructions"] = out
    return json.dumps(m).encode()


_orig_compile_bir_kernel = _bu.compile_bir_kernel


def _patched_compile_bir_kernel(bir_json, tmpdir, neff_name="file.neff"):
    if isinstance(bir_json, str):
        bir_json = bir_json.encode()
    return _orig_compile_bir_kernel(_split_excess_waits(bir_json), tmpdir, neff_name)


if getattr(_bu.compile_bir_kernel, "__name__", "") != "_patched_compile_bir_kernel":
    _bu.compile_bir_kernel = _patched_compile_bir_kernel
    _b2j.compile_bir_kernel = _patched_compile_bir_kernel

# ---------------------------------------------------------------------------


def _build_nc():
    nc = bass.Bass()
    xT = nc.dram_tensor("xT", [D, L], FP, kind="ExternalInput")
    wihT = nc.dram_tensor("wihT", [D, G], FP, kind="ExternalInput")
    whhT = nc.dram_tensor("whhT", [D, G], FP, kind="ExternalInput")
    ident = nc.dram_tensor("ident", [128, 128], FP, kind="ExternalInput")
    brz = nc.dram_tensor("brz", [128, 8], FP, kind="ExternalInput")
    bihn = nc.dram_tensor("bihn", [128, 4], FP, kind="ExternalInput")
    bhhn = nc.dram_tensor("bhhn", [128, 4], FP, kind="ExternalInput")
    out = nc.dram_tensor("out", [D, L], FP, kind="ExternalOutput")

    xT_v = xT.rearrange("(c p) t -> p c t", p=128)      # [128, NC, L]
    wih_v = wihT.rearrange("(c p) g -> p c g", p=128)   # [128, NC, G]
    whh_v = whhT.rearrange("(c p) g -> p c g", p=128)
    out_v = out.rearrange("(c p) t -> p c t", p=128)

    Sig = mybir.ActivationFunctionType.Sigmoid
    Tanh = mybir.ActivationFunctionType.Tanh
    Alu = mybir.AluOpType

    with tile.TileContext(nc) as tc:
        with (
            tc.tile_pool(name="const", bufs=1) as cpool,
            tc.tile_pool(name="gx", bufs=1) as gxpool,
            tc.tile_pool(name="hbuf", bufs=1) as hpool,
            tc.tile_pool(name="work", bufs=2) as wpool,
            tc.tile_pool(name="work1", bufs=1) as wpool1,
            tc.tile_pool(name="psum", bufs=8, space="PSUM") as ppool,
        ):
            # --- constants into SBUF ---
            wih_sb = cpool.tile([128, NC, G], FP)
            whh_sb = cpool.tile([128, NC, G], FP)
            id_sb = cpool.tile([128, 128], FP)
            brz_sb = cpool.tile([128, 8], FP)
            bihn_sb = cpool.tile([128, 4], FP)
            bhhn_sb = cpool.tile([128, 4], FP)
            xt_sb = cpool.tile([128, NC, L], FP)
            nc.sync.dma_start(out=wih_sb[:], in_=wih_v[:])
            nc.sync.dma_start(out=whh_sb[:], in_=whh_v[:])
            nc.sync.dma_start(out=id_sb[:], in_=ident[:])
            nc.sync.dma_start(out=brz_sb[:], in_=brz[:])
            nc.sync.dma_start(out=bihn_sb[:], in_=bihn[:])
            nc.sync.dma_start(out=bhhn_sb[:], in_=bhhn[:])
            nc.sync.dma_start(out=xt_sb[:], in_=xT_v[:])

            # --- gx = W_ih @ x, stored [128, MC, K-1+L]; first 6 cols zero ---
            PADL = K - 1
            gx = gxpool.tile([128, MC, PADL + L], FP)
            nc.gpsimd.memset(gx[:, :, 0:PADL], 0.0)
            for mi in range(MC):
                for ni in range(NT):
                    ps = ppool.tile([128, TCH], FP, tag="ps")
                    for c in range(NC):
                        nc.tensor.matmul(
                            ps[:],
                            wih_sb[:, c, mi * 128:(mi + 1) * 128],
                            xt_sb[:, c, ni * TCH:(ni + 1) * TCH],
                            start=(c == 0), stop=(c == NC - 1),
                        )
                    dst = gx[:, mi, PADL + ni * TCH: PADL + (ni + 1) * TCH]
                    if (mi + ni) % 2 == 0:
                        nc.scalar.copy(dst, ps[:])
                    else:
                        nc.vector.tensor_copy(dst, ps[:])

            # --- recurrence ---
            h_cur = hpool.tile([128, NC, L], FP, tag="h0")
            h_nxt = hpool.tile([128, NC, L], FP, tag="h1")

            for k in range(K):
                src_h, dst_h = (h_cur, h_nxt) if k % 2 == 0 else (h_nxt, h_cur)
                for ni in range(NT):
                    t0 = ni * TCH
                    gofs = k + t0  # gx column offset for this step/chunk
                    r_t = wpool.tile([128, NC, TCH], FP, tag="r")
                    z_t = wpool.tile([128, NC, TCH], FP, tag="z")
                    n_t = wpool.tile([128, NC, TCH], FP, tag="n")
                    t_t = wpool1.tile([128, NC, TCH], FP, tag="t")
                    if k == 0:
                        # h=0: gates from gx directly
                        for j in range(NC):
                            nc.scalar.activation(
                                r_t[:, j, :], gx[:, j, gofs:gofs + TCH],
                                Sig, bias=brz_sb[:, j:j + 1])
                            nc.scalar.activation(
                                z_t[:, j, :], gx[:, 4 + j, gofs:gofs + TCH],
                                Sig, bias=brz_sb[:, 4 + j:5 + j])
                            nc.vector.tensor_scalar(
                                t_t[:, j, :], r_t[:, j, :],
                                bhhn_sb[:, j:j + 1], None, Alu.mult)
                        nc.vector.tensor_tensor(
                            t_t[:], t_t[:], gx[:, 8:12, gofs:gofs + TCH], Alu.add)
                        for j in range(NC):
                            nc.scalar.activation(
                                n_t[:, j, :], t_t[:, j, :],
                                Tanh, bias=bihn_sb[:, j:j + 1])
                        # h1 = (1-z)*n = n - z*n
                        nc.vector.tensor_tensor(
                            t_t[:], z_t[:], n_t[:], Alu.mult)
                        nc.vector.tensor_tensor(
                            dst_h[:, :, t0:t0 + TCH], n_t[:], t_t[:], Alu.subtract)
                        continue

                    # r and z gates: psum = W_hh@h + gx (identity matmul), sigmoid
                    for gi, gate_t in ((0, r_t), (1, z_t)):
                        for j in range(NC):
                            mi = gi * 4 + j
                            ps = ppool.tile([128, TCH], FP, tag="ps")
                            for c in range(NC):
                                nc.tensor.matmul(
                                    ps[:],
                                    whh_sb[:, c, mi * 128:(mi + 1) * 128],
                                    src_h[:, c, t0:t0 + TCH],
                                    start=(c == 0), stop=False)
                            nc.tensor.matmul(
                                ps[:], id_sb[:],
                                gx[:, mi, gofs:gofs + TCH],
                                start=False, stop=True)
                            nc.scalar.activation(
                                gate_t[:, j, :], ps[:],
                                Sig, bias=brz_sb[:, mi:mi + 1])
                    # n gate: psum = W_hh@h; t = (psum + b_hh_n)*r; += gx; tanh
                    for j in range(NC):
                        mi = 8 + j
                        ps = ppool.tile([128, TCH], FP, tag="ps")
                        for c in range(NC):
                            nc.tensor.matmul(
                                ps[:],
                                whh_sb[:, c, mi * 128:(mi + 1) * 128],
                                src_h[:, c, t0:t0 + TCH],
                                start=(c == 0), stop=(c == NC - 1))
                        nc.vector.scalar_tensor_tensor(
                            t_t[:, j, :], ps[:], bhhn_sb[:, j:j + 1],
                            r_t[:, j, :], Alu.add, Alu.mult)
                    nc.vector.tensor_tensor(
                        t_t[:], t_t[:], gx[:, 8:12, gofs:gofs + TCH], Alu.add)
                    for j in range(NC):
                        nc.scalar.activation(
                            n_t[:, j, :], t_t[:, j, :],
                            Tanh, bias=bihn_sb[:, j:j + 1])
                    # h' = n + z*(h-n)
                    nc.vector.tensor_tensor(
                        t_t[:], src_h[:, :, t0:t0 + TCH], n_t[:], Alu.subtract)
                    nc.vector.tensor_tensor(t_t[:], z_t[:], t_t[:], Alu.mult)
                    nc.vector.tensor_tensor(
                        dst_h[:, :, t0:t0 + TCH], n_t[:], t_t[:], Alu.add)

            h_fin = h_nxt if K % 2 == 1 else h_cur
            nc.sync.dma_start(out=out_v[:], in_=h_fin[:])
    return nc


_NC_CACHE = None


def kernel(x, W_ih, W_hh, b_ih, b_hh, ksize):
    global _NC_CACHE
    assert int(ksize) == K
    x = np.asarray(x, np.float32)
    W_ih = np.asarray(W_ih, np.float32)
    W_hh = np.asarray(W_hh, np.float32)
    b_ih = np.asarray(b_ih, np.float32)
    b_hh = np.asarray(b_hh, np.float32)

    wihT = np.ascontiguousarray(W_ih.T)           # [D, G]
    whhT = np.ascontiguousarray(W_hh.T)
    ident = np.eye(128, dtype=np.float32)
    # per-partition bias layouts: chunk j of gate dim -> column j
    bsum = (b_ih + b_hh).reshape(MC, 128).T       # [128, 12]
    brz = np.ascontiguousarray(bsum[:, 0:8])
    bihn = np.ascontiguousarray(b_ih.reshape(MC, 128).T[:, 8:12])
    bhhn = np.ascontiguousarray(b_hh.reshape(MC, 128).T[:, 8:12])

    if _NC_CACHE is None:
        _NC_CACHE = _build_nc()
    nc = _NC_CACHE

    in_maps = []
    for b in range(B):
        xT = np.ascontiguousarray(x[b].T)         # [D, L]
        in_maps.append({
            "xT": xT, "wihT": wihT, "whhT": whhT, "ident": ident,
            "brz": brz, "bihn": bihn, "bhhn": bhhn,
        })
    res = _bu.run_bass_kernel_spmd(nc, in_maps, core_ids=list(range(B)))
    out = np.stack([res.results[b]["out"].T for b in range(B)])  # [B, L, D]
    return np.ascontiguousarray(out.astype(np.float32))


# revision 6
# speedup vs baseline: 3.3927x; 3.3927x over previous
"""LocalRNN (sliding-window GRU) Trainium2 Bass kernel.

x: [8, 1024, 512] f32, window K=7, GRU hidden 512. Output [8, 1024, 512].

Strategy: one batch element per NeuronCore (8 cores). Per core, feature-major
layout [feature, token]: the K-step recurrence needs h as matmul rhs in
[d, tok] layout, and gate elementwise ops run on [128, ntok] tiles. The
input projection gx = W_ih @ x is computed once for all tokens; window step k
reads gx shifted by k columns (left zero-padding becomes 6 zero columns).
"""

import json
import numpy as np
import ml_dtypes

import concourse.bass as bass
import concourse.mybir as mybir
import concourse.tile as tile
from concourse import bass_utils as _bu
from concourse import bass2jax as _b2j

B, L, D, K = 8, 1024, 512, 7
G = 3 * D          # 1536 gate dim
NC = D // 128      # 4 feature chunks
MC = G // 128      # 12 gate chunks
NT = 2             # token chunks of 512
TCH = L // NT      # 512 tokens per chunk
FP = mybir.dt.float32
BF = mybir.dt.bfloat16

# ---------------------------------------------------------------------------
# Workaround: this walrus build rejects instructions with >2 semaphore waits
# ("Too many sync wait commands"). Split excess waits onto injected NoOps on
# the same engine immediately before the instruction.
def _split_excess_waits(bir: bytes) -> bytes:
    m = json.loads(bir)
    n_new = 0
    for fn in m.get("functions", []):
        for blk in fn.get("blocks", []):
            insts = blk.get("instructions", [])
            out = []
            for inst in insts:
                si = inst.get("sync_info")
                waits = si.get("on_wait") if si else None
                lim = 1
                if waits and len(waits) > lim:
                    extra = waits[:-lim]
                    inst["sync_info"]["on_wait"] = waits[-lim:]
                    for i in range(0, len(extra), 1):
                        n_new += 1
                        out.append({
                            "debug": inst.get("debug", 0),
                            "engine": inst["engine"],
                            "ins": [],
                            "name": f"{inst['name']}w{n_new}",
                            "opcode": "NoOp",
                            "outs": [],
                            "sync_info": {
                                "on_update": [],
                                "on_wait": extra[i:i + 1],
                            },
                        })
                out.append(inst)
            blk["instructions"] = out
    return json.dumps(m).encode()


_orig_compile_bir_kernel = _bu.compile_bir_kernel


def _patched_compile_bir_kernel(bir_json, tmpdir, neff_name="file.neff"):
    if isinstance(bir_json, str):
        bir_json = bir_json.encode()
    return _orig_compile_bir_kernel(_split_excess_waits(bir_json), tmpdir, neff_name)


if getattr(_bu.compile_bir_kernel, "__name__", "") != "_patched_compile_bir_kernel":
    _bu.compile_bir_kernel = _patched_compile_bir_kernel
    _b2j.compile_bir_kernel = _patched_compile_bir_kernel

# ---------------------------------------------------------------------------


def _build_nc():
    nc = bass.Bass()
    xT = nc.dram_tensor("xT", [D, L], BF, kind="ExternalInput")
    wihT = nc.dram_tensor("wihT", [D, G], BF, kind="ExternalInput")
    whhT = nc.dram_tensor("whhT", [D, G], BF, kind="ExternalInput")
    ident = nc.dram_tensor("ident", [128, 128], BF, kind="ExternalInput")
    brz = nc.dram_tensor("brz", [128, 8], FP, kind="ExternalInput")
    bihn = nc.dram_tensor("bihn", [128, 4], FP, kind="ExternalInput")
    bhhn = nc.dram_tensor("bhhn", [128, 4], FP, kind="ExternalInput")
    out = nc.dram_tensor("out", [D, L], BF, kind="ExternalOutput")

    xT_v = xT.rearrange("(c p) t -> p c t", p=128)      # [128, NC, L]
    wih_v = wihT.rearrange("(c p) g -> p c g", p=128)   # [128, NC, G]
    whh_v = whhT.rearrange("(c p) g -> p c g", p=128)
    out_v = out.rearrange("(c p) t -> p c t", p=128)

    Sig = mybir.ActivationFunctionType.Sigmoid
    Tanh = mybir.ActivationFunctionType.Tanh
    Alu = mybir.AluOpType

    with tile.TileContext(nc) as tc:
        with (
            tc.tile_pool(name="const", bufs=1) as cpool,
            tc.tile_pool(name="gx", bufs=1) as gxpool,
            tc.tile_pool(name="hbuf", bufs=1) as hpool,
            tc.tile_pool(name="work", bufs=2) as wpool,
            tc.tile_pool(name="work1", bufs=1) as wpool1,
            tc.tile_pool(name="psum", bufs=8, space="PSUM") as ppool,
        ):
            # --- constants into SBUF ---
            wih_sb = cpool.tile([128, NC, G], BF)
            whh_sb = cpool.tile([128, NC, G], BF)
            id_sb = cpool.tile([128, 128], BF)
            brz_sb = cpool.tile([128, 8], FP)
            bihn_sb = cpool.tile([128, 4], FP)
            bhhn_sb = cpool.tile([128, 4], FP)
            xt_sb = cpool.tile([128, NC, L], BF)
            nc.sync.dma_start(out=wih_sb[:], in_=wih_v[:])
            nc.sync.dma_start(out=whh_sb[:], in_=whh_v[:])
            nc.sync.dma_start(out=id_sb[:], in_=ident[:])
            nc.sync.dma_start(out=brz_sb[:], in_=brz[:])
            nc.sync.dma_start(out=bihn_sb[:], in_=bihn[:])
            nc.sync.dma_start(out=bhhn_sb[:], in_=bhhn[:])
            nc.sync.dma_start(out=xt_sb[:], in_=xT_v[:])

            # --- gx = W_ih @ x, stored [128, MC, K-1+L]; first 6 cols zero ---
            PADL = K - 1
            gx = gxpool.tile([128, MC, PADL + L], BF)
            nc.gpsimd.memset(gx[:, :, 0:PADL], 0.0)
            for mi in range(MC):
                for ni in range(NT):
                    ps = ppool.tile([128, TCH], FP, tag="ps")
                    for c in range(NC):
                        nc.tensor.matmul(
                            ps[:],
                            wih_sb[:, c, mi * 128:(mi + 1) * 128],
                            xt_sb[:, c, ni * TCH:(ni + 1) * TCH],
                            start=(c == 0), stop=(c == NC - 1),
                        )
                    dst = gx[:, mi, PADL + ni * TCH: PADL + (ni + 1) * TCH]
                    if (mi + ni) % 2 == 0:
                        nc.scalar.copy(dst, ps[:])
                    else:
                        nc.vector.tensor_copy(dst, ps[:])

            # --- recurrence ---
            h_cur = hpool.tile([128, NC, L], BF, tag="h0")
            h_nxt = hpool.tile([128, NC, L], BF, tag="h1")

            for k in range(K):
                src_h, dst_h = (h_cur, h_nxt) if k % 2 == 0 else (h_nxt, h_cur)
                for ni in range(NT):
                    t0 = ni * TCH
                    gofs = k + t0  # gx column offset for this step/chunk
                    r_t = wpool.tile([128, NC, TCH], BF, tag="r")
                    z_t = wpool.tile([128, NC, TCH], BF, tag="z")
                    n_t = wpool.tile([128, NC, TCH], BF, tag="n")
                    t_t = wpool1.tile([128, NC, TCH], BF, tag="t")
                    if k == 0:
                        # h=0: gates from gx directly
                        for j in range(NC):
                            nc.scalar.activation(
                                r_t[:, j, :], gx[:, j, gofs:gofs + TCH],
                                Sig, bias=brz_sb[:, j:j + 1])
                            nc.scalar.activation(
                                z_t[:, j, :], gx[:, 4 + j, gofs:gofs + TCH],
                                Sig, bias=brz_sb[:, 4 + j:5 + j])
                            nc.vector.tensor_scalar(
                                t_t[:, j, :], r_t[:, j, :],
                                bhhn_sb[:, j:j + 1], None, Alu.mult)
                        nc.vector.tensor_tensor(
                            t_t[:], t_t[:], gx[:, 8:12, gofs:gofs + TCH], Alu.add)
                        for j in range(NC):
                            nc.scalar.activation(
                                n_t[:, j, :], t_t[:, j, :],
                                Tanh, bias=bihn_sb[:, j:j + 1])
                        # h1 = (1-z)*n = n - z*n
                        nc.vector.tensor_tensor(
                            t_t[:], z_t[:], n_t[:], Alu.mult)
                        nc.vector.tensor_tensor(
                            dst_h[:, :, t0:t0 + TCH], n_t[:], t_t[:], Alu.subtract)
                        continue

                    # r and z gates: psum = W_hh@h + gx (identity matmul), sigmoid
                    for gi, gate_t in ((0, r_t), (1, z_t)):
                        for j in range(NC):
                            mi = gi * 4 + j
                            ps = ppool.tile([128, TCH], FP, tag="ps")
                            for c in range(NC):
                                nc.tensor.matmul(
                                    ps[:],
                                    whh_sb[:, c, mi * 128:(mi + 1) * 128],
                                    src_h[:, c, t0:t0 + TCH],
                                    start=(c == 0), stop=False)
                            nc.tensor.matmul(
                                ps[:], id_sb[:],
                                gx[:, mi, gofs:gofs + TCH],
                                start=False, stop=True)
                            nc.scalar.activation(
                                gate_t[:, j, :], ps[:],
                                Sig, bias=brz_sb[:, mi:mi + 1])
                    # n gate: psum = W_hh@h; t = (psum + b_hh_n)*r; += gx; tanh
                    for j in range(NC):
                        mi = 8 + j
                        ps = ppool.tile([128, TCH], FP, tag="ps")
                        for c in range(NC):
                            nc.tensor.matmul(
                                ps[:],
                                whh_sb[:, c, mi * 128:(mi + 1) * 128],
                                src_h[:, c, t0:t0 + TCH],
                                start=(c == 0), stop=(c == NC - 1))
                        nc.vector.scalar_tensor_tensor(
                            t_t[:, j, :], ps[:], bhhn_sb[:, j:j + 1],
                            r_t[:, j, :], Alu.add, Alu.mult)
                    nc.vector.tensor_tensor(
                        t_t[:], t_t[:], gx[:, 8:12, gofs:gofs + TCH], Alu.add)
                    for j in range(NC):
                        nc.scalar.activation(
                            n_t[:, j, :], t_t[:, j, :],
                            Tanh, bias=bihn_sb[:, j:j + 1])
                    # h' = n + z*(h-n)
                    nc.vector.tensor_tensor(
                        t_t[:], src_h[:, :, t0:t0 + TCH], n_t[:], Alu.subtract)
                    nc.vector.tensor_tensor(t_t[:], z_t[:], t_t[:], Alu.mult)
                    nc.vector.tensor_tensor(
                        dst_h[:, :, t0:t0 + TCH], n_t[:], t_t[:], Alu.add)

            h_fin = h_nxt if K % 2 == 1 else h_cur
            nc.sync.dma_start(out=out_v[:], in_=h_fin[:])
    return nc


def _make_in_maps(x, W_ih, W_hh, b_ih, b_hh):
    wihT = np.ascontiguousarray(W_ih.T).astype(ml_dtypes.bfloat16)  # [D, G]
    whhT = np.ascontiguousarray(W_hh.T).astype(ml_dtypes.bfloat16)
    ident = np.eye(128, dtype=np.float32).astype(ml_dtypes.bfloat16)
    # per-partition bias layouts: chunk j of gate dim -> column j
    bsum = (b_ih + b_hh).reshape(MC, 128).T.astype(np.float32)  # [128, 12]
    brz = np.ascontiguousarray(bsum[:, 0:8])
    bihn = np.ascontiguousarray(b_ih.reshape(MC, 128).T[:, 8:12]).astype(np.float32)
    bhhn = np.ascontiguousarray(b_hh.reshape(MC, 128).T[:, 8:12]).astype(np.float32)
    in_maps = []
    for b in range(B):
        xT = np.ascontiguousarray(x[b].T).astype(ml_dtypes.bfloat16)  # [D, L]
        in_maps.append({
            "xT": xT, "wihT": wihT, "whhT": whhT, "ident": ident,
            "brz": brz, "bihn": bihn, "bhhn": bhhn,
        })
    return in_maps


_NC_CACHE = None


def kernel(x, W_ih, W_hh, b_ih, b_hh, ksize):
    global _NC_CACHE
    assert int(ksize) == K
    x = np.asarray(x, np.float32)
    W_ih = np.asarray(W_ih, np.float32)
    W_hh = np.asarray(W_hh, np.float32)
    b_ih = np.asarray(b_ih, np.float32)
    b_hh = np.asarray(b_hh, np.float32)

    if _NC_CACHE is None:
        _NC_CACHE = _build_nc()
    nc = _NC_CACHE

    in_maps = _make_in_maps(x, W_ih, W_hh, b_ih, b_hh)
    res = _bu.run_bass_kernel_spmd(nc, in_maps, core_ids=list(range(B)))
    out = np.stack([np.asarray(res.results[b]["out"], np.float32).T for b in range(B)])  # [B, L, D]
    return np.ascontiguousarray(out.astype(np.float32))


# revision 8
# speedup vs baseline: 3.4397x; 1.0139x over previous
"""LocalRNN (sliding-window GRU) Trainium2 Bass kernel.

x: [8, 1024, 512] f32, window K=7, GRU hidden 512. Output [8, 1024, 512].

Strategy: one batch element per NeuronCore (8 cores). Per core, feature-major
layout [feature, token]: the K-step recurrence needs h as matmul rhs in
[d, tok] layout, and gate elementwise ops run on [128, ntok] tiles. The
input projection gx = W_ih @ x is computed once for all tokens; window step k
reads gx shifted by k columns (left zero-padding becomes 6 zero columns).
"""

import json
import numpy as np
import ml_dtypes

import concourse.bass as bass
import concourse.mybir as mybir
import concourse.tile as tile
from concourse import bass_utils as _bu
from concourse import bass2jax as _b2j

B, L, D, K = 8, 1024, 512, 7
G = 3 * D          # 1536 gate dim
NC = D // 128      # 4 feature chunks
MC = G // 128      # 12 gate chunks
NT = 2             # token chunks of 512
TCH = L // NT      # 512 tokens per chunk
FP = mybir.dt.float32
BF = mybir.dt.bfloat16

# ---------------------------------------------------------------------------
# Workaround: this walrus build rejects instructions with >2 semaphore waits
# ("Too many sync wait commands"). Split excess waits onto injected NoOps on
# the same engine immediately before the instruction.
def _split_excess_waits(bir: bytes) -> bytes:
    m = json.loads(bir)
    n_new = 0
    for fn in m.get("functions", []):
        for blk in fn.get("blocks", []):
            insts = blk.get("instructions", [])
            out = []
            for inst in insts:
                si = inst.get("sync_info")
                waits = si.get("on_wait") if si else None
                lim = 1
                if waits and len(waits) > lim:
                    extra = waits[:-lim]
                    inst["sync_info"]["on_wait"] = waits[-lim:]
                    for i in range(0, len(extra), 1):
                        n_new += 1
                        out.append({
                            "debug": inst.get("debug", 0),
                            "engine": inst["engine"],
                            "ins": [],
                            "name": f"{inst['name']}w{n_new}",
                            "opcode": "NoOp",
                            "outs": [],
                            "sync_info": {
                                "on_update": [],
                                "on_wait": extra[i:i + 1],
                            },
                        })
                out.append(inst)
            blk["instructions"] = out
    return json.dumps(m).encode()


_orig_compile_bir_kernel = _bu.compile_bir_kernel


def _patched_compile_bir_kernel(bir_json, tmpdir, neff_name="file.neff"):
    if isinstance(bir_json, str):
        bir_json = bir_json.encode()
    return _orig_compile_bir_kernel(_split_excess_waits(bir_json), tmpdir, neff_name)


if getattr(_bu.compile_bir_kernel, "__name__", "") != "_patched_compile_bir_kernel":
    _bu.compile_bir_kernel = _patched_compile_bir_kernel
    _b2j.compile_bir_kernel = _patched_compile_bir_kernel

# ---------------------------------------------------------------------------


def _build_nc():
    nc = bass.Bass()
    xT = nc.dram_tensor("xT", [D, L], BF, kind="ExternalInput")
    wihT = nc.dram_tensor("wihT", [D, G], BF, kind="ExternalInput")
    whhT = nc.dram_tensor("whhT", [D, G], BF, kind="ExternalInput")
    ident = nc.dram_tensor("ident", [128, 128], BF, kind="ExternalInput")
    brz = nc.dram_tensor("brz", [128, 8], FP, kind="ExternalInput")
    bihn = nc.dram_tensor("bihn", [128, 4], FP, kind="ExternalInput")
    bhhn = nc.dram_tensor("bhhn", [128, 4], FP, kind="ExternalInput")
    out = nc.dram_tensor("out", [D, L], BF, kind="ExternalOutput")

    xT_v = xT.rearrange("(c p) t -> p c t", p=128)      # [128, NC, L]
    wih_v = wihT.rearrange("(c p) g -> p c g", p=128)   # [128, NC, G]
    whh_v = whhT.rearrange("(c p) g -> p c g", p=128)
    out_v = out.rearrange("(c p) t -> p c t", p=128)

    Sig = mybir.ActivationFunctionType.Sigmoid
    Tanh = mybir.ActivationFunctionType.Tanh
    Alu = mybir.AluOpType

    with tile.TileContext(nc) as tc:
        with (
            tc.tile_pool(name="const", bufs=1) as cpool,
            tc.tile_pool(name="gx", bufs=1) as gxpool,
            tc.tile_pool(name="hbuf", bufs=1) as hpool,
            tc.tile_pool(name="work", bufs=3) as wpool,
            tc.tile_pool(name="work1", bufs=2) as wpool1,
            tc.tile_pool(name="psum", bufs=8, space="PSUM") as ppool,
        ):
            # --- constants into SBUF ---
            wih_sb = cpool.tile([128, NC, G], BF)
            whh_sb = cpool.tile([128, NC, G], BF)
            id_sb = cpool.tile([128, 128], BF)
            brz_sb = cpool.tile([128, 8], FP)
            bihn_sb = cpool.tile([128, 4], FP)
            bhhn_sb = cpool.tile([128, 4], FP)
            xt_sb = cpool.tile([128, NC, L], BF)
            nc.sync.dma_start(out=wih_sb[:], in_=wih_v[:])
            nc.sync.dma_start(out=whh_sb[:], in_=whh_v[:])
            nc.sync.dma_start(out=id_sb[:], in_=ident[:])
            nc.sync.dma_start(out=brz_sb[:], in_=brz[:])
            nc.sync.dma_start(out=bihn_sb[:], in_=bihn[:])
            nc.sync.dma_start(out=bhhn_sb[:], in_=bhhn[:])
            nc.sync.dma_start(out=xt_sb[:], in_=xT_v[:])

            # --- gx = W_ih @ x, stored [128, MC, K-1+L]; first 6 cols zero ---
            PADL = K - 1
            gx = gxpool.tile([128, MC, PADL + L], BF)
            nc.gpsimd.memset(gx[:, :, 0:PADL], 0.0)
            for j in range(NC):
                nc.vector.tensor_scalar(
                    gx[:, 8 + j, 0:PADL], gx[:, 8 + j, 0:PADL],
                    bihn_sb[:, j:j + 1], None, Alu.add)
            for mi in range(MC):
                for ni in range(NT):
                    ps = ppool.tile([128, TCH], FP, tag="ps")
                    for c in range(NC):
                        nc.tensor.matmul(
                            ps[:],
                            wih_sb[:, c, mi * 128:(mi + 1) * 128],
                            xt_sb[:, c, ni * TCH:(ni + 1) * TCH],
                            start=(c == 0), stop=(c == NC - 1),
                        )
                    dst = gx[:, mi, PADL + ni * TCH: PADL + (ni + 1) * TCH]
                    if mi < 8:
                        nc.vector.tensor_copy(dst, ps[:])
                    else:
                        nc.scalar.add(dst, ps[:], bihn_sb[:, mi - 8:mi - 7])

            # --- recurrence ---
            h_a = [hpool.tile([128, NC, TCH], BF, tag=f"ha{ni}", name=f"ha{ni}") for ni in range(NT)]
            h_b = [hpool.tile([128, NC, TCH], BF, tag=f"hb{ni}", name=f"hb{ni}") for ni in range(NT)]

            for k in range(K):
                src_l, dst_l = (h_a, h_b) if k % 2 == 0 else (h_b, h_a)
                for ni in range(NT):
                    t0 = ni * TCH
                    src_h, dst_h = src_l[ni], dst_l[ni]
                    gofs = k + t0  # gx column offset for this step/chunk
                    r_t = wpool.tile([128, NC, TCH], BF, tag="r")
                    z_t = wpool.tile([128, NC, TCH], BF, tag="z")
                    n_t = wpool.tile([128, NC, TCH], BF, tag="n")
                    t_t = wpool1.tile([128, NC, TCH], BF, tag="t")
                    if k == 0:
                        # h=0: gates from gx directly
                        for j in range(NC):
                            nc.scalar.activation(
                                r_t[:, j, :], gx[:, j, gofs:gofs + TCH],
                                Sig, bias=brz_sb[:, j:j + 1])
                            nc.scalar.activation(
                                z_t[:, j, :], gx[:, 4 + j, gofs:gofs + TCH],
                                Sig, bias=brz_sb[:, 4 + j:5 + j])
                            nc.vector.tensor_scalar(
                                t_t[:, j, :], r_t[:, j, :],
                                bhhn_sb[:, j:j + 1], None, Alu.mult)
                        nc.vector.tensor_tensor(
                            t_t[:], t_t[:], gx[:, 8:12, gofs:gofs + TCH], Alu.add)
                        nc.scalar.activation(n_t[:], t_t[:], Tanh)
                        # h1 = (1-z)*n = n - z*n
                        nc.vector.tensor_tensor(
                            t_t[:], z_t[:], n_t[:], Alu.mult)
                        nc.vector.tensor_tensor(
                            dst_h[:], n_t[:], t_t[:], Alu.subtract)
                        continue

                    # r and z gates: psum = W_hh@h + gx (identity matmul), sigmoid
                    for gi, gate_t in ((0, r_t), (1, z_t)):
                        for j in range(NC):
                            mi = gi * 4 + j
                            ps = ppool.tile([128, TCH], FP, tag="ps")
                            for c in range(NC):
                                nc.tensor.matmul(
                                    ps[:],
                                    whh_sb[:, c, mi * 128:(mi + 1) * 128],
                                    src_h[:, c, :],
                                    start=(c == 0), stop=False)
                            nc.tensor.matmul(
                                ps[:], id_sb[:],
                                gx[:, mi, gofs:gofs + TCH],
                                start=False, stop=True)
                            nc.scalar.activation(
                                gate_t[:, j, :], ps[:],
                                Sig, bias=brz_sb[:, mi:mi + 1])
                    # n gate: psum = W_hh@h; t = (psum + b_hh_n)*r; += gx; tanh
                    for j in range(NC):
                        mi = 8 + j
                        ps = ppool.tile([128, TCH], FP, tag="ps")
                        for c in range(NC):
                            nc.tensor.matmul(
                                ps[:],
                                whh_sb[:, c, mi * 128:(mi + 1) * 128],
                                src_h[:, c, :],
                                start=(c == 0), stop=(c == NC - 1))
                        nc.vector.scalar_tensor_tensor(
                            t_t[:, j, :], ps[:], bhhn_sb[:, j:j + 1],
                            r_t[:, j, :], Alu.add, Alu.mult)
                    nc.vector.tensor_tensor(
                        t_t[:], t_t[:], gx[:, 8:12, gofs:gofs + TCH], Alu.add)
                    nc.scalar.activation(n_t[:], t_t[:], Tanh)
                    # h' = n + z*(h-n)
                    nc.vector.tensor_tensor(
                        t_t[:], src_h[:], n_t[:], Alu.subtract)
                    nc.vector.tensor_tensor(t_t[:], z_t[:], t_t[:], Alu.mult)
                    nc.vector.tensor_tensor(
                        dst_h[:], n_t[:], t_t[:], Alu.add)

            fin_l = h_b if K % 2 == 1 else h_a
            for ni in range(NT):
                nc.sync.dma_start(
                    out=out_v[:, :, ni * TCH:(ni + 1) * TCH], in_=fin_l[ni][:])
    return nc


def _make_in_maps(x, W_ih, W_hh, b_ih, b_hh):
    wihT = np.ascontiguousarray(W_ih.T).astype(ml_dtypes.bfloat16)  # [D, G]
    whhT = np.ascontiguousarray(W_hh.T).astype(ml_dtypes.bfloat16)
    ident = np.eye(128, dtype=np.float32).astype(ml_dtypes.bfloat16)
    # per-partition bias layouts: chunk j of gate dim -> column j
    bsum = (b_ih + b_hh).reshape(MC, 128).T.astype(np.float32)  # [128, 12]
    brz = np.ascontiguousarray(bsum[:, 0:8])
    bihn = np.ascontiguousarray(b_ih.reshape(MC, 128).T[:, 8:12]).astype(np.float32)
    bhhn = np.ascontiguousarray(b_hh.reshape(MC, 128).T[:, 8:12]).astype(np.float32)
    in_maps = []
    for b in range(B):
        xT = np.ascontiguousarray(x[b].T).astype(ml_dtypes.bfloat16)  # [D, L]
        in_maps.append({
            "xT": xT, "wihT": wihT, "whhT": whhT, "ident": ident,
            "brz": brz, "bihn": bihn, "bhhn": bhhn,
        })
    return in_maps


_NC_CACHE = None


def kernel(x, W_ih, W_hh, b_ih, b_hh, ksize):
    global _NC_CACHE
    assert int(ksize) == K
    x = np.asarray(x, np.float32)
    W_ih = np.asarray(W_ih, np.float32)
    W_hh = np.asarray(W_hh, np.float32)
    b_ih = np.asarray(b_ih, np.float32)
    b_hh = np.asarray(b_hh, np.float32)

    if _NC_CACHE is None:
        _NC_CACHE = _build_nc()
    nc = _NC_CACHE

    in_maps = _make_in_maps(x, W_ih, W_hh, b_ih, b_hh)
    res = _bu.run_bass_kernel_spmd(nc, in_maps, core_ids=list(range(B)))
    out = np.stack([np.asarray(res.results[b]["out"], np.float32).T for b in range(B)])  # [B, L, D]
    return np.ascontiguousarray(out.astype(np.float32))
